# revision 14
# baseline (speedup 1.0000x reference)
"""Trainium2 Bass kernel for the fern/sparse-table CTE model.

Strategy: data-parallel over batch N=32 across 8 cores (4 images each).
Front-end uses a packed-key top-2 ambiguous-bit selection: |z| is scaled by
2^16 inside the Abs activation, magic-rounded to integers via
one abs_max+2^23 ALU op, and the bit index is packed into the low 4 bits
(key = q + k/16, exact in f32). Two min-reductions then give both ambiguous
bits AND their |z| values; sigmoid runs only on the two selected values.
Thresholds and side-2 negation are folded into the host-built window slabs.
The T=4 table-row gather is served by one dma_gather per (image, fern) from
a host-prebuilt pair table (256B elements holding the 4 candidate rows).
Votes are weighted and reduced on DVE; 2x2 average pooling and the
classifier run as PE matmuls.
"""
import os
import numpy as np
from contextlib import ExitStack

import concourse.bacc as bacc
import concourse.bass as bass
import concourse.tile as tile
from concourse import mybir
from concourse.bass_utils import run_bass_kernel_spmd

F32 = mybir.dt.float32
I32 = mybir.dt.int32
I16 = mybir.dt.int16
ALU = mybir.AluOpType
ACT = mybir.ActivationFunctionType

M, K, L = 8, 10, 6
D = 16                      # D_OUT
NCLS = 10
N, C, H, W = 32, 3, 64, 64
NCORES = 8
NI = N // NCORES            # images per core
NPX = H * W                 # 4096
NT = NPX // 128             # 32 pixel tiles per image
TM = NT * M                 # 256
NPAIR = 45
PTROWS = NPAIR * 256        # 11520 elements per fern
SC = 65536.0                # |z| quantization scale (2^16)
MAGIC = float(2 ** 23)
MASKBIG = float(2 ** 22)


def _build_pair_table(table: np.ndarray) -> np.ndarray:
    """PT[m, pid*256+base8, 64] f32; rows j=ilo+2*ihi of the 256B element are
    table[m*1024 + unpack(base8;klo,khi) + ilo*2^klo + ihi*2^khi]."""
    tbl = table.reshape(M, 1024, D)
    PT = np.zeros((M, PTROWS, 4 * D), dtype=np.float32)
    base8 = np.arange(256)
    for khi in range(K):
        for klo in range(khi):
            pid = khi * (khi - 1) // 2 + klo
            rest = [k for k in range(K) if k not in (klo, khi)]
            unpacked = np.zeros(256, dtype=np.int64)
            for r, k in enumerate(rest):
                unpacked += ((base8 >> r) & 1) << k
            for ihi in range(2):
                for ilo in range(2):
                    j = ilo + 2 * ihi
                    rows = unpacked + ilo * (1 << klo) + ihi * (1 << khi)
                    PT[:, pid * 256 + base8, j * D:(j + 1) * D] = tbl[:, rows, :]
    return PT


def _host_windows(x, c1, c2, dy1, dx1, dy2, dx2, thresholds):
    """Pack the 2*M*K shifted LxL-anchored windows per image into contiguous
    slabs. Thresholds are folded into side 1 and side 2 is negated so the
    on-chip op is a single add: z = w1 + w2 = v1 - v2 - thr."""
    xp = np.pad(x, ((0, 0), (0, 0), (0, L - 1), (0, L - 1)))
    N_ = x.shape[0]
    xw = np.empty((N_, 2, M * K, NPX), dtype=np.float32)
    for m in range(M):
        for k in range(K):
            r = m * K + k
            xw[:, 0, r] = (xp[:, c1[m, k], dy1[m, k]:dy1[m, k] + H,
                              dx1[m, k]:dx1[m, k] + W].reshape(N_, NPX)
                           - thresholds[m, k])
            xw[:, 1, r] = -(xp[:, c2[m, k], dy2[m, k]:dy2[m, k] + H,
                               dx2[m, k]:dx2[m, k] + W].reshape(N_, NPX))
    return xw


def _host_consts(w_pred, b_pred):
    ident = np.eye(128, dtype=np.float32)
    # iotapow[:, 0:160]: (idx%10)/16 ; [:, 160:320]: 2^(idx%10)
    kpat = np.arange(160) % 10
    iotapow = np.zeros((128, 322), dtype=np.float32)
    iotapow[:, 0:160] = (kpat / 16.0)[None, :]
    iotapow[:, 160:320] = (2.0 ** kpat)[None, :]
    iotapow[:, 320] = MAGIC
    onesi = np.zeros((128, 2), dtype=np.int32)
    onesi[:, 0] = 1
    onesi[:, 1] = 15
    # pool lhsT: poolW[p, s*128 + s2*32 + w2] = 0.25 if s2==s and (p%64)//2==w2
    poolW = np.zeros((128, 4, 4, 32), dtype=np.float32)
    p = np.arange(128)
    for s in range(4):
        poolW[p, s, s, (p % 64) // 2] = 0.25
    poolW = poolW.reshape(128, 512)
    # classifier lhsT: wqT[p, c, cls] = w_pred[cls, d*1024 + (4g+s)*32 + w2]
    # with c = d*8+g, p = s*32+w2
    wq = w_pred.reshape(NCLS, D, 8, 4, 32)          # [cls, d, g, s, w2]
    wqT = np.transpose(wq, (3, 4, 1, 2, 0)).reshape(128, D * 8, NCLS)
    wqT = np.ascontiguousarray(wqT.reshape(128, D * 8 * NCLS)).astype(np.float32)
    bpred = b_pred.reshape(NCLS, 1).astype(np.float32)
    return ident, iotapow, onesi, poolW, wqT, bpred


def _build_kernel(c1, c2, dy1, dx1, dy2, dx2):
    nc = bacc.Bacc("TRN2", num_devices=NCORES, num_swdge_queues=4)

    xw_p = nc.declare_dram_parameter("xw", [NI, 2, M * K, NPX], F32, isOutput=False)
    pt_p = nc.declare_dram_parameter("pt", [M, PTROWS, 4 * D], F32, isOutput=False)
    id_p = nc.declare_dram_parameter("ident", [128, 128], F32, isOutput=False)
    ip_p = nc.declare_dram_parameter("iotapow", [128, 322], F32, isOutput=False)
    oi_p = nc.declare_dram_parameter("onesi", [128, 2], I32, isOutput=False)
    pw_p = nc.declare_dram_parameter("poolw", [128, 512], F32, isOutput=False)
    wq_p = nc.declare_dram_parameter("wqt", [128, D * 8 * NCLS], F32, isOutput=False)
    bp_p = nc.declare_dram_parameter("bpred", [NCLS, 1], F32, isOutput=False)
    out_p = nc.declare_dram_parameter("out", [NCLS, NI], F32, isOutput=True)

    with tile.TileContext(nc, num_cores=NCORES) as tc:
        with ExitStack() as ctx:
            cpool = ctx.enter_context(tc.tile_pool(name="consts", bufs=1))
            colp = ctx.enter_context(tc.tile_pool(name="col", bufs=1))
            bpxp = ctx.enter_context(tc.tile_pool(name="bpx", bufs=2))
            kp = ctx.enter_context(tc.tile_pool(name="kstage", bufs=1))
            sm = ctx.enter_context(tc.tile_pool(name="smalls", bufs=1))
            wp = ctx.enter_context(tc.tile_pool(name="wgidx", bufs=2))
            idxp = ctx.enter_context(tc.tile_pool(name="idx", bufs=2))
            vp = ctx.enter_context(tc.tile_pool(name="votes", bufs=8))
            fp = ctx.enter_context(tc.tile_pool(name="feat", bufs=1))
            flp = ctx.enter_context(tc.tile_pool(name="flat", bufs=1))
            tps = ctx.enter_context(tc.tile_pool(name="tpsum", bufs=2, space="PSUM"))
            dps = ctx.enter_context(tc.tile_pool(name="dpsum", bufs=1, space="PSUM"))
            pps = ctx.enter_context(tc.tile_pool(name="ppsum", bufs=2, space="PSUM"))
            lps = ctx.enter_context(tc.tile_pool(name="lpsum", bufs=1, space="PSUM"))

            # ---- constants ----
            ident = cpool.tile([128, 128], F32)
            nc.sync.dma_start(ident[:], id_p.ap())
            iotapow = cpool.tile([128, 322], F32)
            nc.sync.dma_start(iotapow[:], ip_p.ap())
            onesi = cpool.tile([128, 2], I32)
            nc.sync.dma_start(onesi[:], oi_p.ap())
            poolw = cpool.tile([128, 512], F32)
            nc.sync.dma_start(poolw[:], pw_p.ap())
            wqt = cpool.tile([128, D * 8 * NCLS], F32)
            nc.sync.dma_start(wqt[:], wq_p.ap())
            bpred = cpool.tile([NCLS, 1], F32)
            nc.sync.dma_start(bpred[:], bp_p.ap())

            # broadcast views of the K-periodic constants: [128, 16, 16, 10]
            iota16v = iotapow[:, 0:160].rearrange(
                "p (b k) -> p b k", b=16, k=K).unsqueeze(1) \
                .broadcast_to([128, 16, 16, K])
            pow2v = iotapow[:, 160:320].rearrange(
                "p (b k) -> p b k", b=16, k=K).unsqueeze(1) \
                .broadcast_to([128, 16, 16, K])
            onesb = onesi[:, 0:1].broadcast_to([128, TM])
            c15b = onesi[:, 1:2].broadcast_to([128, TM])

            feat = fp.tile([128, NI, NT, D], F32)
            flatbuf = flp.tile([128, D, 8, NI], F32)

            v_tiles: dict = {}
            wt_tiles: dict = {}

            def stage_back(i):
                # weighted vote reduce (in-place on the gathered tiles) for
                # image i, then 2x2 avg pool via PE.
                wti = wt_tiles.pop(i)
                for m in range(M):
                    v = v_tiles.pop((i, m))
                    wb = wti[:, m, :, :].rearrange("p t j -> p (t j)") \
                        .unsqueeze(-1).broadcast_to([128, NT * 4, D])
                    vv = v[:].rearrange("p t j d -> p (t j) d")
                    nc.vector.tensor_tensor(vv, vv, wb, ALU.mult)
                    nc.vector.tensor_add(
                        v[:, :, 0:2, :].rearrange("p t j d -> p t (j d)"),
                        v[:, :, 0:2, :].rearrange("p t j d -> p t (j d)"),
                        v[:, :, 2:4, :].rearrange("p t j d -> p t (j d)"))
                    nc.vector.tensor_add(v[:, :, 0, :], v[:, :, 0, :],
                                         v[:, :, 1, :])
                    if m == 0:
                        nc.scalar.copy(feat[:, i], v[:, :, 0, :])
                    else:
                        nc.vector.tensor_add(feat[:, i], feat[:, i],
                                             v[:, :, 0, :])
                pps_t = pps.tile([128, 8, D], F32, tag="pp")
                for g in range(8):
                    for s in range(4):
                        nc.tensor.matmul(
                            pps_t[:, g, :],
                            poolw[:, s * 128:(s + 1) * 128],
                            feat[:, i, 4 * g + s, :],
                            start=(s == 0), stop=(s == 3))
                nc.scalar.copy(
                    flatbuf[:, :, :, i],
                    pps_t[:].rearrange("p g d -> p d g"))

            def load_windows(i):
                b1 = colp.tile([M * K, NPX], F32, tag="b1")
                b2 = colp.tile([M * K, NPX], F32, tag="b2")
                nc.sync.dma_start(b1[:], xw_p.ap()[i, 0])
                nc.scalar.dma_start(b2[:], xw_p.ap()[i, 1])
                return b1, b2

            nxt_win = load_windows(0)
            for img in range(NI):
                # ---- stage A: z = w1 + w2 (thr folded on host), column ----
                b1, b2 = nxt_win
                nc.vector.tensor_add(b1[:], b1[:], b2[:])          # z

                # ---- transposes to pixel layout, scaled by SC ----
                zpx = bpxp.tile([128, NT, M, K], F32, tag="bpx")
                done = 0
                while done < NT:
                    grp = min(6, NT - done)
                    tp = tps.tile([128, 480], F32, tag="tp")
                    for i in range(grp):
                        t_ = done + i
                        nc.tensor.transpose(
                            tp[:, i * 80:(i + 1) * 80],
                            b1[:, t_ * 128:(t_ + 1) * 128],
                            ident[0:M * K, 0:M * K])
                    nc.scalar.copy(
                        zpx[:, done:done + grp, :, :].rearrange("p t m k -> p (t m k)"),
                        tp[:, 0:80 * grp])
                    done += grp

                # ---- stage B: packed-key top-2 + base ([128, TM, K]) ----
                tkey = kp.tile([128, TM, K], F32, tag="tkey")
                zpxF = zpx[:].rearrange("p t m k -> p (t m k)")
                zpx3 = zpx[:].rearrange("p t m k -> p (t m) k")
                zpx4 = zpx[:].rearrange("p (a c) m k -> p a (c m) k", a=16, c=2)
                tkeyF = tkey[:].rearrange("p s k -> p (s k)")
                tkey4 = tkey[:].rearrange("p (a b) k -> p a b k", a=16, b=16)

                def small(tag, dt=F32):
                    return sm.tile([128, TM], dt, tag=tag, name=tag)

                key1 = small("key1")
                key2 = small("key2")
                base_f = small("base_f")

                def bcm(t):  # [128,TM] -> broadcast over K
                    return t[:].unsqueeze(-1).broadcast_to([128, TM, K])

                # tkey = |z*SC| + 2^23  (magic round to integer grid)
                nc.scalar.activation(tkeyF, zpxF, ACT.Abs, scale=SC)
                nc.scalar.activation(tkeyF, tkeyF, ACT.Identity,
                                     bias=iotapow[:, 320:321])
                # key = (tkey - 2^23) + k/16
                nc.vector.scalar_tensor_tensor(tkey4, tkey4, MAGIC, iota16v,
                                               ALU.subtract, ALU.add)
                nc.vector.tensor_reduce(key1[:], tkey[:], mybir.AxisListType.X,
                                        ALU.min)
                eq = kp.tile([128, TM, K], F32, tag="eq")
                eqF = eq[:].rearrange("p s k -> p (s k)")
                nc.vector.tensor_tensor(eq[:], tkey[:], bcm(key1), ALU.is_equal)
                nc.vector.scalar_tensor_tensor(eqF, eqF, MASKBIG, tkeyF,
                                               ALU.mult, ALU.add)
                nc.vector.tensor_reduce(key2[:], eq[:], mybir.AxisListType.X,
                                        ALU.min)
                # base = sum_k (z>0) * 2^k
                hp2 = kp.tile([128, TM, K], F32, tag="eq")
                hp24 = hp2[:].rearrange("p (a b) k -> p a b k", a=16, b=16)
                nc.vector.scalar_tensor_tensor(hp24, zpx4, 0.0, pow2v,
                                               ALU.is_gt, ALU.mult)
                nc.vector.tensor_reduce(base_f[:], hp2[:], mybir.AxisListType.X,
                                        ALU.add)

                # ---- stage C: extraction / pair / word / weight smalls ----
                # key*16 = q*16 + k (exact int); k = &15, q = >>4
                ks1, ks2 = small("ks1"), small("ks2")
                nc.vector.tensor_scalar(ks1[:], key1[:], 16.0, None, ALU.mult)
                nc.vector.tensor_scalar(ks2[:], key2[:], 16.0, None, ALU.mult)
                ks1i, ks2i = small("ks1i", I32), small("ks2i", I32)
                nc.scalar.copy(ks1i[:], ks1[:])
                nc.scalar.copy(ks2i[:], ks2[:])
                q1i, q2i = small("q1i", I32), small("q2i", I32)
                nc.vector.tensor_scalar(q1i[:], ks1i[:], 4, None,
                                        ALU.arith_shift_right)
                nc.vector.tensor_scalar(q2i[:], ks2i[:], 4, None,
                                        ALU.arith_shift_right)
                # in-place: ks1i becomes k1i, ks1 slot becomes q1
                k1i, k2i = ks1i, ks2i
                nc.vector.tensor_tensor(k1i[:], ks1i[:], c15b, ALU.bitwise_and)
                nc.vector.tensor_tensor(k2i[:], ks2i[:], c15b, ALU.bitwise_and)
                q1, q2 = small("ks1"), small("ks2")
                nc.scalar.copy(q1[:], q1i[:])
                nc.scalar.copy(q2[:], q2i[:])
                basei = small("basei", I32)
                nc.scalar.copy(basei[:], base_f[:])
                kloi, khii = small("kloi", I32), small("khii", I32)
                nc.vector.tensor_tensor(kloi[:], k1i[:], k2i[:], ALU.min)
                nc.vector.tensor_tensor(khii[:], k1i[:], k2i[:], ALU.max)
                swapf = small("swapf")
                nc.vector.tensor_tensor(swapf[:], k1i[:], k2i[:], ALU.is_gt)
                # p2 = 2^k via float exponent-field construction: (k+127)<<23
                p2lo, p2hi = small("p2lo", I32), small("p2hi", I32)
                e1, e2 = small("q1i", I32), small("q2i", I32)
                nc.vector.tensor_scalar(e1[:], kloi[:], 127, 8388608.0,
                                        ALU.add, ALU.mult)
                nc.vector.tensor_scalar(e2[:], khii[:], 127, 8388608.0,
                                        ALU.add, ALU.mult)
                nc.scalar.copy(p2lo[:], e1[:].bitcast(F32))
                nc.scalar.copy(p2hi[:], e2[:].bitcast(F32))
                # qlo/qhi: sort q by bit index
                dlt, qlo, qhi = small("dlt"), small("qlo"), small("qhi")
                nc.vector.tensor_tensor(dlt[:], q2[:], q1[:], ALU.subtract)
                nc.vector.tensor_mul(dlt[:], swapf[:], dlt[:])
                nc.vector.tensor_add(qlo[:], q1[:], dlt[:])
                nc.vector.tensor_add(qhi[:], q1[:], q2[:])
                nc.vector.tensor_tensor(qhi[:], qhi[:], qlo[:], ALU.subtract)
                # hard bits of base at klo/khi; clear them
                lo_and, hi_and = small("lo_and", I32), small("hi_and", I32)
                nc.vector.tensor_tensor(lo_and[:], basei[:], p2lo[:],
                                        ALU.bitwise_and)
                nc.vector.tensor_tensor(hi_and[:], basei[:], p2hi[:],
                                        ALU.bitwise_and)
                nc.vector.tensor_tensor(basei[:], basei[:], lo_and[:],
                                        ALU.subtract)
                nc.vector.tensor_tensor(basei[:], basei[:], hi_and[:],
                                        ALU.subtract)          # base_clear
                hlo, hhi = small("hlo"), small("hhi")
                nc.vector.tensor_scalar(hlo[:], lo_and[:], 0, None, ALU.is_gt)
                nc.vector.tensor_scalar(hhi[:], hi_and[:], 0, None, ALU.is_gt)
                # signed selected z: zsel = (2h-1)*q
                zlo, zhi = small("zlo"), small("zhi")
                nc.vector.tensor_mul(zlo[:], qlo[:], hlo[:])
                nc.vector.scalar_tensor_tensor(zlo[:], zlo[:], 2.0, qlo[:],
                                               ALU.mult, ALU.subtract)
                nc.vector.tensor_mul(zhi[:], qhi[:], hhi[:])
                nc.vector.scalar_tensor_tensor(zhi[:], zhi[:], 2.0, qhi[:],
                                               ALU.mult, ALU.subtract)
                blo, bhi = small("blo"), small("bhi")
                nc.scalar.activation(blo[:], zlo[:], ACT.Sigmoid, scale=1.0 / SC)
                nc.scalar.activation(bhi[:], zhi[:], ACT.Sigmoid, scale=1.0 / SC)
                clo, chi = small("clo"), small("chi")
                nc.vector.tensor_scalar(clo[:], blo[:], -1.0, 1.0,
                                        ALU.mult, ALU.add)
                nc.vector.tensor_scalar(chi[:], bhi[:], -1.0, 1.0,
                                        ALU.mult, ALU.add)
                # weights, m-major: [128, M, NT, 4]
                wt = wp.tile([128, M, NT, 4], F32, tag="wt")

                def wslot(jj):  # [128, M, NT] view ordered as (t, m)
                    return wt[:, :, :, jj].rearrange("p m t -> p t m")

                def v3(t):  # [128,TM] -> [128, NT, M]
                    return t[:].rearrange("p (t m) -> p t m", t=NT, m=M)

                nc.vector.tensor_mul(wslot(0), v3(clo), v3(chi))
                nc.vector.tensor_mul(wslot(1), v3(blo), v3(chi))
                nc.vector.tensor_mul(wslot(2), v3(clo), v3(bhi))
                nc.vector.tensor_mul(wslot(3), v3(blo), v3(bhi))
                # pid = khi*(khi-1)/2 + klo
                pidi = small("pidi", I32)
                nc.vector.tensor_tensor(pidi[:], khii[:], khii[:],
                                        ALU.mult)
                nc.vector.tensor_tensor(pidi[:], pidi[:], khii[:], ALU.subtract)
                nc.vector.tensor_scalar(pidi[:], pidi[:], 1, None,
                                        ALU.arith_shift_right)
                nc.vector.tensor_tensor(pidi[:], pidi[:], kloi[:], ALU.add)
                # base8 = pack(base_clear; p2lo, p2hi)
                loi2, hii2 = small("loi2", I32), small("hii2", I32)
                nc.vector.tensor_scalar(loi2[:], p2lo[:], -1, None, ALU.add)
                nc.vector.tensor_scalar(hii2[:], p2hi[:], -1, None, ALU.add)
                nc.vector.tensor_tensor(loi2[:], basei[:], loi2[:],
                                        ALU.bitwise_and)       # lop
                nc.vector.tensor_tensor(hii2[:], basei[:], hii2[:],
                                        ALU.bitwise_and)       # hip
                t1i, t2i = small("t1i", I32), small("t2i", I32)
                nc.vector.tensor_tensor(t1i[:], hii2[:], loi2[:], ALU.subtract)
                nc.vector.tensor_scalar(t1i[:], t1i[:], 1, None,
                                        ALU.arith_shift_right)
                nc.vector.tensor_tensor(t2i[:], basei[:], hii2[:], ALU.subtract)
                nc.vector.tensor_scalar(t2i[:], t2i[:], 2, None,
                                        ALU.arith_shift_right)
                nc.vector.tensor_tensor(t1i[:], t1i[:], t2i[:], ALU.add)
                nc.vector.tensor_tensor(t1i[:], t1i[:], loi2[:], ALU.add)  # b8
                # gidx = pid*256 + base8
                nc.vector.scalar_tensor_tensor(t1i[:], pidi[:], 256.0, t1i[:],
                                               ALU.mult, ALU.add)
                lowv = small("lowv")
                nc.scalar.copy(lowv[:], t1i[:])

                # ---- stage D: fold gidx [128,(t,m)] -> wrapped idx layout
                # [16, m, t*8+phi] via PE transposes.
                idxt = idxp.tile([128, M, 256], I16, tag="idxt")
                idxv = idxt[0:16, :, :].rearrange("p m (t f) -> p m t f", t=NT, f=8)
                xps = dps.tile([128, 256], F32, tag="xt")
                nc.tensor.transpose(xps[:, 0:128], lowv[:, 0:128], ident)
                nc.tensor.transpose(xps[:, 128:256], lowv[:, 128:256], ident)
                x1s = sm.tile([128, 256], F32, tag="x1s", name="x1s")
                nc.scalar.copy(x1s[:], xps[:])
                for g4 in range(4):
                    yps = dps.tile([16, 512], F32, tag="yt")
                    for ph2 in range(2):
                        phi = g4 * 2 + ph2
                        for a in range(2):
                            nc.tensor.transpose(
                                yps[:, ph2 * 256 + a * 128:ph2 * 256 + a * 128 + 128],
                                x1s[:, a * 128 + phi * 16:a * 128 + phi * 16 + 16],
                                ident)
                        ysl = yps[:, ph2 * 256:(ph2 + 1) * 256] \
                            .rearrange("p (a tt m) -> p m (a tt)", a=2, tt=16, m=M)
                        nc.scalar.copy(idxv[:, :, :, phi], ysl)
                # replicate group 0 into groups 1..7 (tree) -- BEFORE the
                # window prefetch so the gathers aren't blocked behind it.
                nc.sync.dma_start(idxt[16:32, :, :], idxt[0:16, :, :])
                nc.sync.dma_start(idxt[32:64, :, :], idxt[0:32, :, :])
                nc.scalar.dma_start(idxt[64:128, :, :], idxt[0:64, :, :])

                # prefetch next image's windows (queued after the idx DMAs)
                if img + 1 < NI:
                    nxt_win = load_windows(img + 1)

                # ---- stage E: issue gathers; reduce deferred to stage_back ----
                for m in range(M):
                    v = vp.tile([128, NT, 4, D], F32, tag="v")
                    if os.environ.get("KBISECT") == "nogather":
                        nc.vector.memset(v[:].rearrange("p t j d -> p (t j d)"), 0)
                    else:
                        nc.gpsimd.dma_gather(
                            out_ap=v[:].rearrange("p t j d -> p t (j d)"),
                            in_ap=pt_p.ap()[m],
                            idxs_ap=idxt[:, m, :],
                            num_idxs=NPX,
                            num_idxs_reg=NPX,
                            elem_size=4 * D,
                            single_packet=False,
                            queue_num=m % 4,
                        )
                    v_tiles[(img, m)] = v
                wt_tiles[img] = wt
                if img >= 1:
                    stage_back(img - 1)

            stage_back(NI - 1)

            # ---- classifier ----
            lg = lps.tile([NCLS, NI], F32)
            wqv = wqt[:].rearrange("p (c l) -> p c l", c=D * 8, l=NCLS)
            flv = flatbuf[:].rearrange("p d g i -> p (d g) i")
            for c_ in range(D * 8):
                nc.tensor.matmul(lg[:], wqv[:, c_, :], flv[:, c_, :],
                                 start=(c_ == 0), stop=(c_ == D * 8 - 1))
            lsb = flp.tile([NCLS, NI], F32)
            nc.scalar.activation(lsb[:], lg[:], ACT.Identity, bias=bpred[:], scale=1.0)
            nc.sync.dma_start(out_p.ap(), lsb[:])

    nc.compile()
    return nc


def _prep_core_inputs(x, c1, c2, dy1, dx1, dy2, dx2,
                      thresholds, table, w_pred, b_pred):
    xw = _host_windows(np.asarray(x, np.float32), c1, c2, dy1, dx1, dy2, dx2,
                       np.asarray(thresholds, np.float32))
    PT = _build_pair_table(np.asarray(table, np.float32))
    ident, iotapow, onesi, poolW, wqT, bpred = _host_consts(
        np.asarray(w_pred, np.float32), np.asarray(b_pred, np.float32))
    in_maps = []
    for c in range(NCORES):
        in_maps.append(dict(
            xw=np.ascontiguousarray(xw[c * NI:(c + 1) * NI]),
            pt=PT, ident=ident, iotapow=iotapow, onesi=onesi,
            poolw=poolW, wqt=wqT, bpred=bpred,
        ))
    return in_maps


_CACHE: dict = {}


def _get_kernel(c1, c2, dy1, dx1, dy2, dx2):
    key = (c1.tobytes(), c2.tobytes(), dy1.tobytes(), dx1.tobytes(),
           dy2.tobytes(), dx2.tobytes())
    if key not in _CACHE:
        _CACHE[key] = _build_kernel(c1, c2, dy1, dx1, dy2, dx2)
    return _CACHE[key]


def kernel(x, c1, c2, dy1, dx1, dy2, dx2, thresholds, table, w_pred, b_pred):
    x = np.asarray(x, dtype=np.float32)
    c1, c2 = np.asarray(c1, np.int32), np.asarray(c2, np.int32)
    dy1, dx1 = np.asarray(dy1, np.int32), np.asarray(dx1, np.int32)
    dy2, dx2 = np.asarray(dy2, np.int32), np.asarray(dx2, np.int32)
    thresholds = np.asarray(thresholds, np.float32)
    table = np.asarray(table, np.float32)
    w_pred = np.asarray(w_pred, np.float32)
    b_pred = np.asarray(b_pred, np.float32)

    nc = _get_kernel(c1, c2, dy1, dx1, dy2, dx2)

    in_maps = _prep_core_inputs(x, c1, c2, dy1, dx1, dy2, dx2,
                                thresholds, table, w_pred, b_pred)
    res = run_bass_kernel_spmd(nc, in_maps, core_ids=list(range(NCORES)))
    outs = [r["out"].T for r in res.results]      # each [NI, NCLS]
    return np.concatenate(outs, axis=0).astype(np.float32)
